# revision 1
# baseline (speedup 1.0000x reference)
"""MultiHeadAttention (B=4, N=2048, E=1024, H=16) on 8 TRN2 NeuronCores.

Sharding: core c handles batch b = c//2 and head-half hh = c%2 (8 heads,
512 embed dims). Each core computes Q/K/V projections for its 8 heads,
attention, and a partial output projection (contraction over its 512 c-dims).
Host sums the two partials per batch and adds the output bias.

All matmuls run as float32r (tf32 mantissa, fp32 accumulate) at full PE rate.
Layouts are chosen so no transposes are ever needed on device:
  - host ships x.T  [embed, tok] so projections contract embed on partitions
  - Q/K are produced transposed: QT/KT [dout, tok]
  - scores are computed directly as S.T [k, q] (contraction d<=64)
  - V is produced in natural [tok, dv] layout with a ones-column appended per
    head, so attn@V yields O.T [d, q] AND the softmax denominators in row 64
  - softmax skips max-subtraction (|scores/8| < ~3, exp is safe in fp32)
  - output projection consumes O.T directly; host transposes the result once
"""
import sys

sys.path.insert(0, "/opt/trn_rl_repo")

import numpy as np

B, N, E = 4, 2048, 1024
NCORES = 8
HH = 512          # embed dims (8 heads x 64) per core
D = 64
NHEAD = 8         # heads per core

_cache = {}


def _tf32(x):
    u = np.ascontiguousarray(x, dtype=np.float32).view(np.uint32)
    lsb = (u >> 13) & 1
    u = (u + 0x0FFF + lsb) & 0xFFFFE000
    return u.view(np.float32)


def _split_matmul_waits(nc, mybir):
    """fp32r self-loading matmuls cannot carry sync waits (walrus places
    them on the S3_LW struct which has no wait slot). Move every wait off
    Matmult instructions onto InstEventSemaphore instructions inserted
    just before, in block order."""
    n_fixed = 0
    for fn in nc.m.functions:
        for blk in fn.blocks:
            insts = blk.instructions
            i = 0
            while i < len(insts):
                inst = insts[i]
                si = inst.sync_info
                if inst.opcode == "Matmult" and si is not None and len(si.on_wait) > 0:
                    waits = list(si.on_wait)
                    si.on_wait = []
                    inst.sync_info = si
                    pos = i
                    for j in range(0, len(waits), 2):
                        ev = mybir.InstEventSemaphore(
                            name=f"mmgate_{inst.name}_{j}",
                            ins=[],
                            outs=[],
                            sync_info=mybir.SyncInfo(
                                on_wait=waits[j : j + 2], on_update=[]
                            ),
                        )
                        ev.engine = inst.engine
                        nc.register_instruction(ev)
                        insts.insert(pos, ev)
                        pos += 1
                        i += 1
                    n_fixed += 1
                i += 1
            blk.instructions = insts
    return n_fixed


def _build():
    import concourse.mybir as mybir
    import concourse.tile as tile
    import concourse.bacc as bacc

    F32 = mybir.dt.float32
    F32R = mybir.dt.float32r
    EXP = mybir.ActivationFunctionType.Exp

    nc = bacc.Bacc(trn_type="TRN2")

    xtq = nc.dram_tensor("xtq", [E, N], F32R, kind="ExternalInput")
    xtk = nc.dram_tensor("xtk", [E, N], F32R, kind="ExternalInput")
    xtv = nc.dram_tensor("xtv", [E, N], F32R, kind="ExternalInput")
    wqt = nc.dram_tensor("wqt", [E, HH], F32R, kind="ExternalInput")
    wkt = nc.dram_tensor("wkt", [E, HH], F32R, kind="ExternalInput")
    wvt = nc.dram_tensor("wvt", [E, HH], F32R, kind="ExternalInput")
    wot = nc.dram_tensor("wot", [HH, E], F32R, kind="ExternalInput")
    bq = nc.dram_tensor("bq", [HH], F32, kind="ExternalInput")
    bk = nc.dram_tensor("bk", [HH], F32, kind="ExternalInput")
    bv = nc.dram_tensor("bv", [HH], F32, kind="ExternalInput")
    po = nc.dram_tensor("po", [E, N], F32, kind="ExternalOutput")

    with tile.TileContext(nc) as tc:
        with (
            tc.tile_pool(name="consts", bufs=1) as consts,
            tc.tile_pool(name="qk", bufs=1) as qk_pool,
            tc.tile_pool(name="vx", bufs=1) as v_pool,
            tc.tile_pool(name="wo", bufs=1) as wo_pool,
        ):
            # ---------------- constants ----------------
            ones_f = consts.tile([1, 128], F32)
            nc.vector.memset(ones_f, 1.0)
            ones_r = consts.tile([1, 128], F32R)
            nc.vector.tensor_copy(ones_r, ones_f)
            onescol_f = consts.tile([128, NHEAD, 1], F32)
            nc.vector.memset(onescol_f, 1.0)

            bq_t = consts.tile([128, 4], F32)
            bk_t = consts.tile([128, 4], F32)
            nc.sync.dma_start(out=bq_t, in_=bq.ap().rearrange("(t p) -> p t", p=128))
            nc.sync.dma_start(out=bk_t, in_=bk.ap().rearrange("(t p) -> p t", p=128))
            bv_row = consts.tile([1, HH], F32)
            nc.sync.dma_start(out=bv_row, in_=bv.ap().rearrange("(a n) -> a n", a=1))
            bv_row_r = consts.tile([1, HH], F32R)
            nc.vector.tensor_copy(bv_row_r, bv_row)
            bv_bc = consts.tile([128, HH], F32)

            # persistent activations
            QT = [qk_pool.tile([128, N], F32R, tag=f"qt{t}", name=f"qt{t}") for t in range(4)]
            KT = [qk_pool.tile([128, N], F32R, tag=f"kt{t}", name=f"kt{t}") for t in range(4)]
            VE = [v_pool.tile([128, NHEAD, D + 1], F32R, tag=f"ve{g}", name=f"ve{g}") for g in range(16)]
            wo_t = wo_pool.tile([128, 4, E], F32R, tag="wo")

            # ---------------- projections ----------------
            with (
                tc.tile_pool(name="w", bufs=2) as w_pool,
                tc.tile_pool(name="xt", bufs=2) as xt_pool,
                tc.tile_pool(name="pps", bufs=4, space="PSUM") as proj_ps,
            ):
                # broadcast bv to all partitions via K=1 matmul
                bc0 = proj_ps.tile([128, HH], F32, tag="bvbc")
                nc.tensor.matmul(bc0, ones_r, bv_row_r, start=True, stop=True)
                nc.vector.tensor_copy(bv_bc, bc0)

                w_tiles = {}
                for name, wdram in (("q", wqt), ("k", wkt), ("v", wvt)):
                    wt = w_pool.tile([128, 8, HH], F32R, tag="w")
                    nc.sync.dma_start(
                        out=wt, in_=wdram.ap().rearrange("(kt p) n -> p kt n", p=128)
                    )
                    w_tiles[name] = wt

                def qk_proj(xdram, wt, dest, bias_t):
                    for th in range(4):
                        xt = xt_pool.tile([128, 8, 512], F32R, tag="xt")
                        nc.sync.dma_start(
                            out=xt,
                            in_=xdram.ap().rearrange("(kt p) n -> p kt n", p=128)[
                                :, :, 512 * th : 512 * (th + 1)
                            ],
                        )
                        for dt_ in range(4):
                            ps = proj_ps.tile([128, 512], F32, tag="pp")
                            for kt in range(8):
                                nc.tensor.matmul(
                                    ps,
                                    wt[:, kt, 128 * dt_ : 128 * (dt_ + 1)],
                                    xt[:, kt, :],
                                    start=(kt == 0),
                                    stop=(kt == 7),
                                )
                            off = 512 * th
                            nc.vector.tensor_scalar_add(
                                dest[dt_][:, off : off + 512],
                                ps,
                                bias_t[:, dt_ : dt_ + 1],
                            )

                qk_proj(xtq, w_tiles["q"], QT, bq_t)
                qk_proj(xtk, w_tiles["k"], KT, bk_t)

                # V in natural [tok, dv] layout + ones column
                for th in range(4):
                    xt = xt_pool.tile([128, 8, 512], F32R, tag="xt")
                    nc.sync.dma_start(
                        out=xt,
                        in_=xtv.ap().rearrange("(kt p) n -> p kt n", p=128)[
                            :, :, 512 * th : 512 * (th + 1)
                        ],
                    )
                    for tt in range(4):
                        g = 4 * th + tt
                        ps = proj_ps.tile([128, 512], F32, tag="pp")
                        for kt in range(8):
                            nc.tensor.matmul(
                                ps,
                                xt[:, kt, 128 * tt : 128 * (tt + 1)],
                                w_tiles["v"][:, kt, :],
                                start=(kt == 0),
                                stop=(kt == 7),
                            )
                        nc.vector.tensor_add(
                            VE[g][:, :, 0:D],
                            ps.rearrange("p (h d) -> p h d", h=NHEAD),
                            bv_bc.rearrange("p (h d) -> p h d", h=NHEAD),
                        )
                        nc.vector.tensor_copy(VE[g][:, :, D : D + 1], onescol_f)

                # output projection weights (loaded during attention DMA slack)
                nc.sync.dma_start(
                    out=wo_t, in_=wot.ap().rearrange("(ct p) n -> p ct n", p=128)
                )

            # ---------------- attention ----------------
            with (
                tc.tile_pool(name="attn", bufs=5) as attn_pool,
                tc.tile_pool(name="otn", bufs=1) as otn_pool,
                tc.tile_pool(name="small", bufs=2) as small_pool,
                tc.tile_pool(name="ostage", bufs=2) as ostage_pool,
                tc.tile_pool(name="st_ps", bufs=1, space="PSUM") as st_ps,
                tc.tile_pool(name="ot_ps", bufs=2, space="PSUM") as ot_ps,
                tc.tile_pool(name="bc_ps", bufs=1, space="PSUM") as bc_ps,
                tc.tile_pool(name="oj_ps", bufs=1, space="PSUM") as oj_ps,
            ):
                for qb in range(4):
                    q0 = 512 * qb
                    otn = [
                        otn_pool.tile([128, 512], F32R, tag=f"otn{ct}",
                                      name=f"otn{ct}_{qb}")
                        for ct in range(4)
                    ]
                    for h in range(NHEAD):
                        t, par = h // 2, (h % 2) * 64
                        at_tiles = []
                        for g in range(4):
                            stg = st_ps.tile([128, 2048], F32, tag="st")
                            for kg in range(4):
                                kt = 4 * g + kg
                                nc.tensor.matmul(
                                    stg[:, 512 * kg : 512 * (kg + 1)],
                                    KT[t][par : par + 64, 128 * kt : 128 * (kt + 1)],
                                    QT[t][par : par + 64, q0 : q0 + 512],
                                    start=True,
                                    stop=True,
                                )
                            at_g = attn_pool.tile([128, 4, 512], F32R, tag="attnT")
                            nc.scalar.activation(at_g, stg, EXP, scale=0.125)
                            at_tiles.append(at_g)
                        ot = ot_ps.tile([128, 512], F32, tag="ot")
                        for kt in range(16):
                            nc.tensor.matmul(
                                ot[0:65, :],
                                VE[kt][:, h, :],
                                at_tiles[kt // 4][:, kt % 4, :],
                                start=(kt == 0),
                                stop=(kt == 15),
                            )
                        r = small_pool.tile([1, 512], F32R, tag="recip")
                        with nc.allow_low_precision(reason="tf32 softmax denom"):
                            nc.vector.reciprocal(r, ot[64:65, :])
                        bc = bc_ps.tile([128, 512], F32, tag="bc")
                        nc.tensor.matmul(
                            bc[0:64, :], ones_r[:, 0:64], r, start=True, stop=True
                        )
                        rbc = small_pool.tile([64, 512], F32, tag="rbc")
                        nc.vector.tensor_copy(rbc, bc[0:64, :])
                        nc.vector.tensor_mul(
                            otn[t][par : par + 64, :], ot[0:64, :], rbc
                        )
                    # output projection for this q-block
                    for jt in range(8):
                        pj = oj_ps.tile([128, 512], F32, tag="oj")
                        for ct in range(4):
                            nc.tensor.matmul(
                                pj,
                                wo_t[:, ct, 128 * jt : 128 * (jt + 1)],
                                otn[ct],
                                start=(ct == 0),
                                stop=(ct == 3),
                            )
                        oj_sb = ostage_pool.tile([128, 512], F32, tag="oj_sb")
                        nc.vector.tensor_copy(oj_sb, pj)
                        nc.sync.dma_start(
                            out=po.ap()[128 * jt : 128 * (jt + 1), q0 : q0 + 512],
                            in_=oj_sb,
                        )

    nc.compile()
    _split_matmul_waits(nc, mybir)
    return nc


def _get_nc():
    if "nc" not in _cache:
        _cache["nc"] = _build()
    return _cache["nc"]


def kernel(query, key, value, Wq, bq, Wk, bk, Wv, bv, Wo, bo):
    from concourse.bass_utils import run_bass_kernel_spmd

    nc = _get_nc()

    query = np.asarray(query, dtype=np.float32)
    key = np.asarray(key, dtype=np.float32)
    value = np.asarray(value, dtype=np.float32)
    Wq, Wk, Wv, Wo = (np.asarray(w, dtype=np.float32) for w in (Wq, Wk, Wv, Wo))
    bq, bk, bv, bo = (np.asarray(b, dtype=np.float32) for b in (bq, bk, bv, bo))

    in_maps = []
    for c in range(NCORES):
        b, hh = c // 2, c % 2
        cols = slice(HH * hh, HH * (hh + 1))
        in_maps.append(
            {
                "xtq": _tf32(query[b].T),
                "xtk": _tf32(key[b].T),
                "xtv": _tf32(value[b].T),
                "wqt": _tf32(Wq[cols, :].T),
                "wkt": _tf32(Wk[cols, :].T),
                "wvt": _tf32(Wv[cols, :].T),
                "wot": _tf32(Wo[:, cols].T),
                "bq": bq[cols],
                "bk": bk[cols],
                "bv": bv[cols],
            }
        )

    _cache["in_maps"] = in_maps
    res = run_bass_kernel_spmd(nc, in_maps, core_ids=list(range(NCORES)))
    out = np.empty((B, N, E), dtype=np.float32)
    for b in range(B):
        p = res.results[2 * b]["po"] + res.results[2 * b + 1]["po"]
        out[b] = p.T + bo
    return out



# revision 4
# speedup vs baseline: 12.6980x; 12.6980x over previous
"""MultiHeadAttention (B=4, N=2048, E=1024, H=16) on 8 TRN2 NeuronCores.

Sharding: core c handles batch b = c//2 and head-half hh = c%2 (8 heads,
512 embed dims). The wall-clock cost on this setup is dominated by the
axon tunnel (host<->device transfer), so the kernel is built to move the
minimum number of bytes and to reuse a single compiled executable:

  - all activations/weights ship as bf16 (matmuls accumulate in f32)
  - each distinct byte crosses the tunnel exactly once:
      * x[b].T is split between the two cores of a pair and AllGather-ed
        on device ([[0,1],[2,3],[4,5],[6,7]])
      * the per-head-half weight set (wqt,wkt,wvt,wot) is split across
        the 4 cores sharing it and AllGather-ed ([[0,2,4,6],[1,3,5,7]])
  - the two per-pair output partials are summed on device with a
    ReduceScatter, so each core returns a disjoint bf16 quarter of out.T
  - a persistent jax.jit(shard_map) executable is built once; donated
    output buffers are recycled on device between calls (no zero upload)

Compute layout (unchanged from the f32r version, now bf16):
  - host ships x.T  [embed, tok] halves so projections contract embed on
    partitions after the gather
  - Q/K are produced transposed: QT/KT [dout, tok]
  - scores are computed directly as S.T [k, q] (contraction d<=64)
  - V is produced in natural [tok, dv] layout with a ones-column appended
    per head, so attn@V yields O.T [d, q] AND the softmax denominators
  - softmax skips max-subtraction (|scores/8| < ~3, exp is safe in fp32)
  - output projection consumes O.T; partials are pair-summed on device
"""
import sys

sys.path.insert(0, "/opt/trn_rl_repo")

import numpy as np
import ml_dtypes

B, N, E = 4, 2048, 1024
NCORES = 8
HH = 512          # embed dims (8 heads x 64) per core
D = 64
NHEAD = 8         # heads per core
WCHUNK = E * HH   # elements in one weight matrix chunk (1024*512)

PAIRS = [[0, 1], [2, 3], [4, 5], [6, 7]]
QUADS = [[0, 2, 4, 6], [1, 3, 5, 7]]

BF16 = ml_dtypes.bfloat16

_cache = {}


def _split_matmul_waits(nc, mybir):
    """fp32r self-loading matmuls cannot carry sync waits (walrus places
    them on the S3_LW struct which has no wait slot). Move every wait off
    Matmult instructions onto InstEventSemaphore instructions inserted
    just before, in block order."""
    n_fixed = 0
    for fn in nc.m.functions:
        for blk in fn.blocks:
            insts = blk.instructions
            i = 0
            while i < len(insts):
                inst = insts[i]
                si = inst.sync_info
                if inst.opcode == "Matmult" and si is not None and len(si.on_wait) > 0:
                    waits = list(si.on_wait)
                    si.on_wait = []
                    inst.sync_info = si
                    pos = i
                    for j in range(0, len(waits), 2):
                        ev = mybir.InstEventSemaphore(
                            name=f"mmgate_{inst.name}_{j}",
                            ins=[],
                            outs=[],
                            sync_info=mybir.SyncInfo(
                                on_wait=waits[j : j + 2], on_update=[]
                            ),
                        )
                        ev.engine = inst.engine
                        nc.register_instruction(ev)
                        insts.insert(pos, ev)
                        pos += 1
                        i += 1
                    n_fixed += 1
                i += 1
            blk.instructions = insts
    return n_fixed


def _build():
    import concourse.mybir as mybir
    import concourse.tile as tile
    import concourse.bacc as bacc

    F32 = mybir.dt.float32
    F32R = mybir.dt.float32r
    BF = mybir.dt.bfloat16
    EXP = mybir.ActivationFunctionType.Exp

    nc = bacc.Bacc(trn_type="TRN2", num_devices=NCORES)

    xq_h = nc.dram_tensor("xq_h", [HH, N], BF, kind="ExternalInput")
    xk_h = nc.dram_tensor("xk_h", [HH, N], BF, kind="ExternalInput")
    xv_h = nc.dram_tensor("xv_h", [HH, N], BF, kind="ExternalInput")
    wchunk = nc.dram_tensor("wchunk", [WCHUNK], BF, kind="ExternalInput")
    bqkv = nc.dram_tensor("bqkv", [3, HH], F32, kind="ExternalInput")
    po = nc.dram_tensor("po", [HH, N], BF, kind="ExternalOutput")

    with tile.TileContext(nc) as tc:
        with (
            tc.tile_pool(name="dram", bufs=1, space="DRAM") as dram,
            tc.tile_pool(name="consts", bufs=1) as consts,
            tc.tile_pool(name="qk", bufs=1) as qk_pool,
            tc.tile_pool(name="vx", bufs=1) as v_pool,
            tc.tile_pool(name="wo", bufs=1) as wo_pool,
        ):
            # ---------- gather inputs across cores (tunnel dedup) ----------
            xq_b = dram.tile([HH, N], BF)
            xk_b = dram.tile([HH, N], BF)
            xv_b = dram.tile([HH, N], BF)
            xq_g = dram.tile([E, N], BF)
            xk_g = dram.tile([E, N], BF)
            xv_g = dram.tile([E, N], BF)
            w_b = dram.tile([WCHUNK], BF)
            w_g = dram.tile([4, WCHUNK], BF)
            po_full = dram.tile([E, N], F32)
            po_rs = dram.tile([HH, N], F32)

            for src, bounce, gathered in (
                (wchunk, w_b, w_g),
                (xq_h, xq_b, xq_g),
                (xk_h, xk_b, xk_g),
                (xv_h, xv_b, xv_g),
            ):
                nc.gpsimd.dma_start(bounce[:], src.ap())
                nc.gpsimd.collective_compute(
                    "AllGather",
                    mybir.AluOpType.bypass,
                    replica_groups=QUADS if src is wchunk else PAIRS,
                    ins=[bounce[:].opt()],
                    outs=[gathered[:].opt()],
                )

            # ---------------- constants ----------------
            ones_f = consts.tile([1, 128], F32)
            nc.vector.memset(ones_f, 1.0)
            ones_r = consts.tile([1, 128], F32R)
            nc.vector.tensor_copy(ones_r, ones_f)
            ones_b = consts.tile([1, 128], BF)
            nc.vector.tensor_copy(ones_b, ones_f)
            onescol_f = consts.tile([128, NHEAD, 1], F32)
            nc.vector.memset(onescol_f, 1.0)

            bq_t = consts.tile([128, 4], F32)
            bk_t = consts.tile([128, 4], F32)
            nc.sync.dma_start(out=bq_t, in_=bqkv.ap()[0].rearrange("(t p) -> p t", p=128))
            nc.sync.dma_start(out=bk_t, in_=bqkv.ap()[1].rearrange("(t p) -> p t", p=128))
            bv_row = consts.tile([1, HH], F32)
            nc.sync.dma_start(out=bv_row, in_=bqkv.ap()[2].rearrange("(a n) -> a n", a=1))
            bv_row_b = consts.tile([1, HH], BF)
            nc.vector.tensor_copy(bv_row_b, bv_row)
            bv_bc = consts.tile([128, HH], F32)

            # persistent activations
            QT = [qk_pool.tile([128, N], BF, tag=f"qt{t}", name=f"qt{t}") for t in range(4)]
            KT = [qk_pool.tile([128, N], BF, tag=f"kt{t}", name=f"kt{t}") for t in range(4)]
            VE = [v_pool.tile([128, NHEAD, D + 1], BF, tag=f"ve{g}", name=f"ve{g}") for g in range(16)]
            wo_t = wo_pool.tile([128, 4, E], BF, tag="wo")

            # ---------------- projections ----------------
            with (
                tc.tile_pool(name="w", bufs=2) as w_pool,
                tc.tile_pool(name="xt", bufs=2) as xt_pool,
                tc.tile_pool(name="pps", bufs=4, space="PSUM") as proj_ps,
            ):
                # broadcast bv to all partitions via K=1 matmul
                bc0 = proj_ps.tile([128, HH], F32, tag="bvbc")
                nc.tensor.matmul(bc0, ones_b, bv_row_b, start=True, stop=True)
                nc.vector.tensor_copy(bv_bc, bc0)

                w_tiles = {}
                for name, m in (("q", 0), ("k", 1), ("v", 2)):
                    wt = w_pool.tile([128, 8, HH], BF, tag="w", name=f"w{name}")
                    nc.sync.dma_start(
                        out=wt,
                        in_=w_g[:][m].rearrange("(kt p n) -> p kt n", p=128, n=HH),
                    )
                    w_tiles[name] = wt
                nc.sync.dma_start(
                    out=wo_t,
                    in_=w_g[:][3].rearrange("(ct p n) -> p ct n", p=128, n=E),
                )

                def qk_proj(xdram, wt, dest, bias_t):
                    for th in range(4):
                        xt = xt_pool.tile([128, 8, 512], BF, tag="xt", name=f"xt{th}")
                        nc.sync.dma_start(
                            out=xt,
                            in_=xdram[:].rearrange("(kt p) n -> p kt n", p=128)[
                                :, :, 512 * th : 512 * (th + 1)
                            ],
                        )
                        for dt_ in range(4):
                            ps = proj_ps.tile([128, 512], F32, tag="pp")
                            for kt in range(8):
                                nc.tensor.matmul(
                                    ps,
                                    wt[:, kt, 128 * dt_ : 128 * (dt_ + 1)],
                                    xt[:, kt, :],
                                    start=(kt == 0),
                                    stop=(kt == 7),
                                )
                            off = 512 * th
                            nc.vector.tensor_scalar_add(
                                dest[dt_][:, off : off + 512],
                                ps,
                                bias_t[:, dt_ : dt_ + 1],
                            )

                qk_proj(xq_g, w_tiles["q"], QT, bq_t)
                qk_proj(xk_g, w_tiles["k"], KT, bk_t)

                # V in natural [tok, dv] layout + ones column
                for th in range(4):
                    xt = xt_pool.tile([128, 8, 512], BF, tag="xt", name=f"xtv{th}")
                    nc.sync.dma_start(
                        out=xt,
                        in_=xv_g[:].rearrange("(kt p) n -> p kt n", p=128)[
                            :, :, 512 * th : 512 * (th + 1)
                        ],
                    )
                    for tt in range(4):
                        g = 4 * th + tt
                        ps = proj_ps.tile([128, 512], F32, tag="pp")
                        for kt in range(8):
                            nc.tensor.matmul(
                                ps,
                                xt[:, kt, 128 * tt : 128 * (tt + 1)],
                                w_tiles["v"][:, kt, :],
                                start=(kt == 0),
                                stop=(kt == 7),
                            )
                        nc.vector.tensor_add(
                            VE[g][:, :, 0:D],
                            ps.rearrange("p (h d) -> p h d", h=NHEAD),
                            bv_bc.rearrange("p (h d) -> p h d", h=NHEAD),
                        )
                        nc.vector.tensor_copy(VE[g][:, :, D : D + 1], onescol_f)

            # ---------------- attention ----------------
            with (
                tc.tile_pool(name="attn", bufs=5) as attn_pool,
                tc.tile_pool(name="otn", bufs=1) as otn_pool,
                tc.tile_pool(name="small", bufs=2) as small_pool,
                tc.tile_pool(name="ostage", bufs=2) as ostage_pool,
                tc.tile_pool(name="st_ps", bufs=1, space="PSUM") as st_ps,
                tc.tile_pool(name="ot_ps", bufs=2, space="PSUM") as ot_ps,
                tc.tile_pool(name="bc_ps", bufs=1, space="PSUM") as bc_ps,
                tc.tile_pool(name="oj_ps", bufs=1, space="PSUM") as oj_ps,
            ):
                for qb in range(4):
                    q0 = 512 * qb
                    otn = [
                        otn_pool.tile([128, 512], BF, tag=f"otn{ct}",
                                      name=f"otn{ct}_{qb}")
                        for ct in range(4)
                    ]
                    for h in range(NHEAD):
                        t, par = h // 2, (h % 2) * 64
                        at_tiles = []
                        for g in range(4):
                            stg = st_ps.tile([128, 2048], F32, tag="st")
                            for kg in range(4):
                                kt = 4 * g + kg
                                nc.tensor.matmul(
                                    stg[:, 512 * kg : 512 * (kg + 1)],
                                    KT[t][par : par + 64, 128 * kt : 128 * (kt + 1)],
                                    QT[t][par : par + 64, q0 : q0 + 512],
                                    start=True,
                                    stop=True,
                                )
                            at_g = attn_pool.tile([128, 4, 512], BF, tag="attnT")
                            nc.scalar.activation(at_g, stg, EXP, scale=0.125)
                            at_tiles.append(at_g)
                        ot = ot_ps.tile([128, 512], F32, tag="ot")
                        for kt in range(16):
                            nc.tensor.matmul(
                                ot[0:65, :],
                                VE[kt][:, h, :],
                                at_tiles[kt // 4][:, kt % 4, :],
                                start=(kt == 0),
                                stop=(kt == 15),
                            )
                        r = small_pool.tile([1, 512], F32R, tag="recip")
                        with nc.allow_low_precision(reason="tf32 softmax denom"):
                            nc.vector.reciprocal(r, ot[64:65, :])
                        bc = bc_ps.tile([128, 512], F32, tag="bc")
                        nc.tensor.matmul(
                            bc[0:64, :], ones_r[:, 0:64], r, start=True, stop=True
                        )
                        rbc = small_pool.tile([64, 512], F32, tag="rbc")
                        nc.vector.tensor_copy(rbc, bc[0:64, :])
                        nc.vector.tensor_mul(
                            otn[t][par : par + 64, :], ot[0:64, :], rbc
                        )
                    # output projection for this q-block (partial over 512 c-dims)
                    for jt in range(8):
                        pj = oj_ps.tile([128, 512], F32, tag="oj")
                        for ct in range(4):
                            nc.tensor.matmul(
                                pj,
                                wo_t[:, ct, 128 * jt : 128 * (jt + 1)],
                                otn[ct],
                                start=(ct == 0),
                                stop=(ct == 3),
                            )
                        oj_sb = ostage_pool.tile([128, 512], F32, tag="oj_sb")
                        nc.vector.tensor_copy(oj_sb, pj)
                        nc.sync.dma_start(
                            out=po_full[:][128 * jt : 128 * (jt + 1), q0 : q0 + 512],
                            in_=oj_sb,
                        )

                # pair-sum the two partials on device; each core keeps its
                # disjoint half of the summed [E, N] (rank order == hh)
                nc.gpsimd.collective_compute(
                    "ReduceScatter",
                    mybir.AluOpType.add,
                    replica_groups=PAIRS,
                    ins=[po_full[:].opt()],
                    outs=[po_rs[:].opt()],
                )
                for ct in range(4):
                    fin_f = ostage_pool.tile([128, N], F32, tag="fin_f", name=f"ff{ct}")
                    fin_b = ostage_pool.tile([128, N], BF, tag="fin_b", name=f"fb{ct}")
                    nc.sync.dma_start(
                        out=fin_f, in_=po_rs[:][128 * ct : 128 * (ct + 1), :]
                    )
                    nc.vector.tensor_copy(fin_b, fin_f)
                    nc.sync.dma_start(
                        out=po.ap()[128 * ct : 128 * (ct + 1), :], in_=fin_b
                    )

    nc.compile()
    _split_matmul_waits(nc, mybir)
    return nc


class _SpmdRunner:
    """Persistent jax.jit(shard_map) wrapper around the Bass module.

    Built once; donated output buffers are recycled on device between
    calls so only genuinely fresh bytes (the inputs) cross the tunnel.
    """

    def __init__(self, nc, n_cores):
        import jax
        from concourse import bass2jax, mybir
        from jax.experimental.shard_map import shard_map
        from jax.sharding import Mesh, PartitionSpec

        bass2jax.install_neuronx_cc_hook()
        if nc.dbg_addr is not None and nc.dbg_callbacks:
            raise RuntimeError("dbg_callbacks unsupported")
        partition_name = (
            nc.partition_id_tensor.name if nc.partition_id_tensor else None
        )

        in_names, out_names, out_avals, zero_outs = [], [], [], []
        for alloc in nc.m.functions[0].allocations:
            if not isinstance(alloc, mybir.MemoryLocationSet):
                continue
            name = alloc.memorylocations[0].name
            if alloc.kind == "ExternalInput":
                if name != partition_name:
                    in_names.append(name)
            elif alloc.kind == "ExternalOutput":
                out_names.append(name)
                shape = tuple(alloc.tensor_shape)
                dtype = mybir.dt.np(alloc.dtype)
                out_avals.append(jax.core.ShapedArray(shape, dtype))
                zero_outs.append(np.zeros((n_cores * shape[0], *shape[1:]), dtype))
        n_params = len(in_names)
        n_outs = len(out_avals)
        all_in_names = in_names + out_names
        if partition_name is not None:
            all_in_names.append(partition_name)
        self.in_names = in_names
        self.out_names = out_names
        self._init_zero_outs = zero_outs
        self._out_bufs = None

        def _body(*args):
            operands = list(args)
            if partition_name is not None:
                operands.append(bass2jax.partition_id_tensor())
            outs = bass2jax._bass_exec_p.bind(
                *operands,
                out_avals=tuple(out_avals),
                in_names=tuple(all_in_names),
                out_names=tuple(out_names),
                lowering_input_output_aliases=(),
                sim_require_finite=True,
                sim_require_nnan=True,
                nc=nc,
            )
            return tuple(outs)

        import jax as _jax

        devices = _jax.devices()[:n_cores]
        assert len(devices) == n_cores, (
            f"need {n_cores} devices, have {len(_jax.devices())}"
        )
        mesh = Mesh(np.asarray(devices), ("core",))
        self._fn = _jax.jit(
            shard_map(
                _body,
                mesh=mesh,
                in_specs=(PartitionSpec("core"),) * (n_params + n_outs),
                out_specs=(PartitionSpec("core"),) * n_outs,
                check_rep=False,
            ),
            donate_argnums=tuple(range(n_params, n_params + n_outs)),
            keep_unused=True,
        )

    def __call__(self, global_ins):
        args = [global_ins[name] for name in self.in_names]
        bufs = self._out_bufs if self._out_bufs is not None else self._init_zero_outs
        outs = self._fn(*args, *bufs)
        res = {name: np.asarray(o) for name, o in zip(self.out_names, outs)}
        self._out_bufs = list(outs)
        return res


def _get_runner():
    if "runner" not in _cache:
        nc = _build()
        _cache["runner"] = _SpmdRunner(nc, NCORES)
    return _cache["runner"]


def _prepare(query, key, value, Wq, bq, Wk, bk, Wv, bv, Wo):
    """Host-side packing into per-core-concatenated global arrays."""
    def xt_half(x):
        # [B,N,E] f32 -> [B,E,N] bf16 -> [8*512, 2048]; core c slab = rows
        # [c*512, (c+1)*512) = embed dims (c%2)*512.. of batch c//2
        return np.ascontiguousarray(
            np.asarray(x, np.float32).astype(BF16).transpose(0, 2, 1)
        ).reshape(NCORES * HH, N)

    wcat = np.empty((NCORES, WCHUNK), BF16)
    for hh in range(2):
        cols = slice(HH * hh, HH * (hh + 1))
        wcat[0 + hh] = np.ascontiguousarray(Wq[cols, :].T).astype(BF16).reshape(-1)
        wcat[2 + hh] = np.ascontiguousarray(Wk[cols, :].T).astype(BF16).reshape(-1)
        wcat[4 + hh] = np.ascontiguousarray(Wv[cols, :].T).astype(BF16).reshape(-1)
        wcat[6 + hh] = np.ascontiguousarray(Wo[:, cols].T).astype(BF16).reshape(-1)

    bcat = np.empty((NCORES * 3, HH), np.float32)
    for c in range(NCORES):
        cols = slice(HH * (c % 2), HH * (c % 2 + 1))
        bcat[3 * c + 0] = bq[cols]
        bcat[3 * c + 1] = bk[cols]
        bcat[3 * c + 2] = bv[cols]

    return {
        "xq_h": xt_half(query),
        "xk_h": xt_half(key),
        "xv_h": xt_half(value),
        "wchunk": wcat.reshape(NCORES * WCHUNK),
        "bqkv": bcat,
    }


def _finish(po_global, bo):
    # po_global [8*512, 2048] bf16; batch b = rows [b*1024, (b+1)*1024)
    # already in embed order 0..1023 (pair cores emit disjoint halves)
    pt = po_global.reshape(B, E, N).astype(np.float32)
    return np.ascontiguousarray(pt.transpose(0, 2, 1)) + np.asarray(bo, np.float32)


def kernel(query, key, value, Wq, bq, Wk, bk, Wv, bv, Wo, bo):
    runner = _get_runner()
    global_ins = _prepare(query, key, value, Wq, bq, Wk, bk, Wv, bv, Wo)
    _cache["global_ins"] = global_ins
    res = runner(global_ins)
    return _finish(res["po"], bo)
